# revision 26
# baseline (speedup 1.0000x reference)
"""Trainium2 Bass kernel for nn_DemographicParityGap.

reference:
    class_sums[c, s] = sum_{n: bp[n]==c} output[n, s]        # segment sum, [C, S]
    demP = class_sums / output.sum(0)                        # [C, S]
    loss = mean over (c, pairs) of (demP[:, i0] - demP[:, i1])**2
    return -loss

Strategy (data-parallel over the 8 NeuronCores, hint-compliant):
  - Shard N rows across 8 cores.  Each core computes a partial per-(class,
    subgroup) sum; column sums are recovered as class_sums.sum(0) (every row
    belongs to exactly one class), so only one tiny [128, 160] partial per
    core leaves the device.  The host sums the 8 partials (the "all-reduce"
    of the tiny tensor) and finishes the pairwise-gap math.

  Device-side segment sum via one-hot matmuls, batched 16 row-groups per
  matmul so the PE stays off the instruction-issue floor:
    - x layout [128, T*8]: partition p holds rows (p*T + t), t<T, each
      row's 8 subgroup values contiguous.
    - one-hot tile [128, T*10] built by a single DVE is_equal against an
      iota constant packed next to bp in the same preloaded tensor.
    - per 16-group supergroup j: matmul(lhsT = x[:, 128j:128(j+1)] (16
      groups x 8 subgroups), rhs = onehot[:, 160j:160(j+1)] (16 groups x
      10 classes)) -> PSUM [128, 160].  Diagonal 8x10 blocks (t==g) are the
      per-class partial sums; off-diagonal blocks are ignored.  All
      supergroups accumulate into one PSUM tile (start on first, stop on
      last), drained once per core.

  This toolchain's walrus codegen allows exactly ONE sync-wait command per
  instruction (TT/LW/DMA structs alike), which dictates the sync shape:
    - bp+iota preloaded in one DMA; all is_equal ops share that single
      observed dependency.
    - a tiny DVE "observer" copy re-reads the newest one-hot tile before
      each is_equal so the is_equal carries only the PE buffer-release wait.
    - a 1x1 dummy matmul reading only x absorbs the x-DMA wait, so the
      first real matmul of a tile waits only on the DVE one-hot.
    - at most 8 DMAs total (1 bp + NX x-chunks + 1 out), one per DMAHW sem
      lane, so no DMA carries a lane-reuse wait on top of a data wait.
"""

import numpy as np

P = 128          # partitions
C = 10           # num classes
S = 8            # num subgroups
G = 16           # row-groups (of 128 rows each) per matmul; G*S == 128
NCORES = 8

N_FULL = 4_194_304
T = 256          # row-groups per partition per compute tile
NT = 16          # compute tiles per core; R = NT*P*T rows per core
CHUNKS = (1, 1, 2, 4, 8)   # DMA chunk sizes in tiles (sum == NT): small
                           # early chunks start compute fast, big late ones
                           # amortize the ~1.2us HWDGE issue cost per DMA


def build_nc(R, T, NT, chunks):
    """Raw-Bass (no TileContext) pipeline.

    This walrus build allows exactly ONE sync-wait command per instruction;
    Tile's auto-sems routinely embed several (and its tail drain aggregates
    all procs), which fails codegen.  Raw Bass emits every wait as its own
    standalone instruction, which is always legal.

    Engine programs:
      SP (sync):  bp DMA, NX x-chunk DMAs (each -> own sem), final out DMA.
      DVE:        per tile: is_equal one-hot into half of a double buffer,
                  gated on bp DMA and (for reuse) PE tile completions;
                  final PSUM->SBUF drain copy.
      PE:         per tile: J matmuls accumulating into one PSUM tile,
                  gated on the x chunk's DMA sem and the DVE one-hot sem.
    """
    from contextlib import ExitStack

    import concourse.bass as bass
    from concourse import mybir

    assert R == NT * P * T
    assert T % G == 0 and sum(chunks) == NT
    J = T // G
    W = T + C        # packed bp tile width: [bp(T), iota(C)]
    NX = len(chunks)
    offs = [sum(chunks[:k]) for k in range(NX)]        # first tile per chunk
    chunk_of = [k for k in range(NX) for _ in range(chunks[k])]
    f32 = mybir.dt.float32

    nc = bass.Bass()
    x = nc.dram_tensor("x", [R, S], f32, kind="ExternalInput")
    bpk = nc.dram_tensor("bp", [P, NT * W], f32, kind="ExternalInput")
    out = nc.dram_tensor("out", [P, G * C], f32, kind="ExternalOutput")

    # tile i, partition p: rows i*(P*T) + p*T + t  ->  [P, NT, T*S]
    x_r = x[:].rearrange("(i p t) s -> p i (t s)", i=NT, p=P)

    with ExitStack() as ctx:
        x_all = ctx.enter_context(nc.sbuf_tensor([P, NT * T * S], f32))
        bp_all = ctx.enter_context(nc.sbuf_tensor([P, NT * W], f32))
        oh2 = ctx.enter_context(nc.sbuf_tensor([P, 2 * T * C], f32))
        out_sb = ctx.enter_context(nc.sbuf_tensor([P, G * C], f32))
        psum_t = ctx.enter_context(nc.psum_tensor([P, G * C], f32))
        s_bp = [ctx.enter_context(nc.semaphore(f"s_bp{k}")) for k in range(NX)]
        s_x = [ctx.enter_context(nc.semaphore(f"s_x{k}")) for k in range(NX)]
        s_oh = ctx.enter_context(nc.semaphore("s_oh"))
        s_pe = ctx.enter_context(nc.semaphore("s_pe"))
        block = ctx.enter_context(nc.Block(no_gpsimd_drain=True))

        @block.scalar
        def _(scalar):
            # bp chunks on the ACT HWDGE ring, in parallel with x on SP's
            for k, n in enumerate(chunks):
                o = offs[k]
                scalar.dma_start(
                    out=bp_all[:, o * W:(o + n) * W],
                    in_=bpk[:, o * W:(o + n) * W],
                ).then_inc(s_bp[k], 16)

        @block.sync
        def _(sync):
            for k, n in enumerate(chunks):
                o = offs[k]
                sync.dma_start(
                    out=x_all[:, o * T * S:(o + n) * T * S].rearrange(
                        "p (i w) -> p i w", i=n),
                    in_=x_r[:, o:o + n, :],
                ).then_inc(s_x[k], 16)
            sync.wait_ge(s_oh, NT + 1)
            sync.dma_start(out=out[:], in_=out_sb[:]).then_inc(s_bp[0], 16)

        @block.vector
        def _(vector):
            for i in range(NT):
                if i == offs[chunk_of[i]]:
                    vector.wait_ge(s_bp[chunk_of[i]], 16)
                if i >= 2:
                    # oh half (i % 2) is reused: wait for tile i-2's matmuls
                    vector.wait_ge(s_pe, i - 1)
                bp_ap = bp_all[:, i * W:i * W + T]
                bp_bcast = bass.AP(
                    tensor=bp_ap.tensor,
                    offset=bp_ap.offset,
                    ap=[bp_ap.ap[0], [bp_ap.ap[1][0], T], [0, C]],
                )
                io_ap = bp_all[:, i * W + T:i * W + T + C]
                io_bcast = bass.AP(
                    tensor=io_ap.tensor,
                    offset=io_ap.offset,
                    ap=[io_ap.ap[0], [0, T], io_ap.ap[1]],
                )
                half = (i % 2) * T * C
                oh3 = oh2[:, half:half + T * C].rearrange(
                    "p (t c) -> p t c", t=T, c=C)
                vector.tensor_tensor(
                    out=oh3, in0=bp_bcast, in1=io_bcast,
                    op=mybir.AluOpType.is_equal,
                ).then_inc(s_oh, 1)
            vector.wait_ge(s_pe, NT)
            vector.tensor_copy(out=out_sb[:], in_=psum_t[:]).then_inc(s_oh, 1)

        @block.tensor
        def _(tensor):
            for i in range(NT):
                if i == offs[chunk_of[i]]:
                    tensor.wait_ge(s_x[chunk_of[i]], 16)
                tensor.wait_ge(s_oh, i + 1)
                xcol = i * T * S
                half = (i % 2) * T * C
                for j in range(J):
                    first = i == 0 and j == 0
                    last = i == NT - 1 and j == J - 1
                    mm = tensor.matmul(
                        out=psum_t[:],
                        lhsT=x_all[:, xcol + j * (G * S):
                                   xcol + (j + 1) * (G * S)],
                        rhs=oh2[:, half + j * (G * C):
                                half + (j + 1) * (G * C)],
                        start=first, stop=last,
                    )
                    if j == J - 1:
                        mm.then_inc(s_pe, 1)
    return nc


_CACHE = {}


def _get_nc(R, T, NT, chunks):
    key = (R, T, NT, tuple(chunks))
    if key not in _CACHE:
        _CACHE[key] = build_nc(R, T, NT, chunks)
    return _CACHE[key]


def pack_bp(bpf_shard, T, NT):
    """[R] f32 -> [P, NT*(T+C)] f32 matching the x layout.

    x slot (p, i*T + t) holds row i*(P*T) + p*T + t; bp uses the same
    permutation, with iota(C) appended per compute tile.
    """
    R = bpf_shard.shape[0]
    assert R == NT * P * T
    perm = bpf_shard.reshape(NT, P, T).transpose(1, 0, 2)
    out = np.empty((P, NT, T + C), np.float32)
    out[:, :, :T] = perm
    out[:, :, T:] = np.arange(C, dtype=np.float32)
    return np.ascontiguousarray(out.reshape(P, NT * (T + C)))


def finish_host(partials):
    """partials: list of [P, G*C] f32 per-core PSUM drains -> scalar loss."""
    acc = np.zeros((P, G * C), np.float64)
    for r in partials:
        acc += r.astype(np.float64)
    cs_T = np.zeros((S, C), np.float64)
    for j in range(G):
        cs_T += acc[j * S:(j + 1) * S, j * C:(j + 1) * C]
    class_sums = cs_T.T                      # [C, S]
    colsum = class_sums.sum(axis=0)          # == output.sum(0)
    demP = class_sums / colsum
    i0, i1 = np.triu_indices(S, k=1)
    dpgs = (demP[:, i0] - demP[:, i1]) ** 2
    loss = dpgs.sum() / (C * i0.shape[0])
    return np.asarray(-loss, dtype=np.float32)


def run_device(x, bpf, trace=False, **trace_kwargs):
    """x: [N, 8] f32, bpf: [N] f32 (integer-valued). Returns BassKernelResults."""
    from concourse.bass_utils import run_bass_kernel_spmd

    N = x.shape[0]
    assert N % (NCORES * P * T) == 0, N
    R = N // NCORES
    NT_ = R // (P * T)
    in_maps = [
        {"x": x[c * R:(c + 1) * R],
         "bp": pack_bp(bpf[c * R:(c + 1) * R], T, NT_)}
        for c in range(NCORES)
    ]
    nc = _get_nc(R, T, NT_, CHUNKS)
    return run_bass_kernel_spmd(
        nc, in_maps, core_ids=list(range(NCORES)), trace=trace, **trace_kwargs
    )


def kernel(output, biased_predictions, labels=None, num_classes=10,
           num_subgroups=8, **_ignored):
    assert int(num_classes) == C and int(num_subgroups) == S
    x = np.ascontiguousarray(np.asarray(output), dtype=np.float32)
    bp = np.asarray(biased_predictions)
    bpf = np.ascontiguousarray(bp.astype(np.float32))
    res = run_device(x, bpf)
    return finish_host([r["out"] for r in res.results])


# revision 27
# speedup vs baseline: 1.0492x; 1.0492x over previous
"""Trainium2 Bass kernel for nn_DemographicParityGap.

reference:
    class_sums[c, s] = sum_{n: bp[n]==c} output[n, s]        # segment sum, [C, S]
    demP = class_sums / output.sum(0)                        # [C, S]
    loss = mean over (c, pairs) of (demP[:, i0] - demP[:, i1])**2
    return -loss

Strategy (data-parallel over the 8 NeuronCores, hint-compliant):
  - Shard N rows across 8 cores.  Each core computes a partial per-(class,
    subgroup) sum; column sums are recovered as class_sums.sum(0) (every row
    belongs to exactly one class), so only one tiny [128, 160] partial per
    core leaves the device.  The host sums the 8 partials (the "all-reduce"
    of the tiny tensor) and finishes the pairwise-gap math.

  Device-side segment sum via one-hot matmuls, batched 16 row-groups per
  matmul so the PE stays off the instruction-issue floor:
    - x layout [128, T*8]: partition p holds rows (p*T + t), t<T, each
      row's 8 subgroup values contiguous.
    - one-hot tile [128, T*10] built by a single DVE is_equal against an
      iota constant packed next to bp in the same preloaded tensor.
    - per 16-group supergroup j: matmul(lhsT = x[:, 128j:128(j+1)] (16
      groups x 8 subgroups), rhs = onehot[:, 160j:160(j+1)] (16 groups x
      10 classes)) -> PSUM [128, 160].  Diagonal 8x10 blocks (t==g) are the
      per-class partial sums; off-diagonal blocks are ignored.  All
      supergroups accumulate into one PSUM tile (start on first, stop on
      last), drained once per core.

  This toolchain's walrus codegen allows exactly ONE sync-wait command per
  instruction (TT/LW/DMA structs alike), which dictates the sync shape:
    - bp+iota preloaded in one DMA; all is_equal ops share that single
      observed dependency.
    - a tiny DVE "observer" copy re-reads the newest one-hot tile before
      each is_equal so the is_equal carries only the PE buffer-release wait.
    - a 1x1 dummy matmul reading only x absorbs the x-DMA wait, so the
      first real matmul of a tile waits only on the DVE one-hot.
    - at most 8 DMAs total (1 bp + NX x-chunks + 1 out), one per DMAHW sem
      lane, so no DMA carries a lane-reuse wait on top of a data wait.
"""

import numpy as np

P = 128          # partitions
C = 10           # num classes
S = 8            # num subgroups
G = 16           # row-groups (of 128 rows each) per matmul; G*S == 128
NCORES = 8

N_FULL = 4_194_304
T = 256          # row-groups per partition per compute tile
NT = 16          # compute tiles per core; R = NT*P*T rows per core
CHUNKS = (1,) * 16         # DMA chunk sizes in tiles (sum == NT).  Per-tile
                           # chunks keep PE gating fine-grained; the ~1.2us
                           # HWDGE issue cost per DMA overlaps the data
                           # stream (2.9us/MiB), and bp DMAs ride the
                           # scalar-engine ring in parallel.


def build_nc(R, T, NT, chunks):
    """Raw-Bass (no TileContext) pipeline.

    This walrus build allows exactly ONE sync-wait command per instruction;
    Tile's auto-sems routinely embed several (and its tail drain aggregates
    all procs), which fails codegen.  Raw Bass emits every wait as its own
    standalone instruction, which is always legal.

    Engine programs:
      SP (sync):  bp DMA, NX x-chunk DMAs (each -> own sem), final out DMA.
      DVE:        per tile: is_equal one-hot into half of a double buffer,
                  gated on bp DMA and (for reuse) PE tile completions;
                  final PSUM->SBUF drain copy.
      PE:         per tile: J matmuls accumulating into one PSUM tile,
                  gated on the x chunk's DMA sem and the DVE one-hot sem.
    """
    from contextlib import ExitStack

    import concourse.bass as bass
    from concourse import mybir

    assert R == NT * P * T
    assert T % G == 0 and sum(chunks) == NT
    J = T // G
    W = T + C        # packed bp tile width: [bp(T), iota(C)]
    NX = len(chunks)
    offs = [sum(chunks[:k]) for k in range(NX)]        # first tile per chunk
    chunk_of = [k for k in range(NX) for _ in range(chunks[k])]
    f32 = mybir.dt.float32

    nc = bass.Bass()
    x = nc.dram_tensor("x", [R, S], f32, kind="ExternalInput")
    bpk = nc.dram_tensor("bp", [P, NT * W], f32, kind="ExternalInput")
    out = nc.dram_tensor("out", [P, G * C], f32, kind="ExternalOutput")

    # tile i, partition p: rows i*(P*T) + p*T + t  ->  [P, NT, T*S]
    x_r = x[:].rearrange("(i p t) s -> p i (t s)", i=NT, p=P)

    with ExitStack() as ctx:
        x_all = ctx.enter_context(nc.sbuf_tensor([P, NT * T * S], f32))
        bp_all = ctx.enter_context(nc.sbuf_tensor([P, NT * W], f32))
        oh2 = ctx.enter_context(nc.sbuf_tensor([P, 2 * T * C], f32))
        out_sb = ctx.enter_context(nc.sbuf_tensor([P, G * C], f32))
        psum_t = ctx.enter_context(nc.psum_tensor([P, G * C], f32))
        s_bp = [ctx.enter_context(nc.semaphore(f"s_bp{k}")) for k in range(NX)]
        s_x = [ctx.enter_context(nc.semaphore(f"s_x{k}")) for k in range(NX)]
        s_oh = ctx.enter_context(nc.semaphore("s_oh"))
        s_pe = ctx.enter_context(nc.semaphore("s_pe"))
        block = ctx.enter_context(nc.Block(no_gpsimd_drain=True))

        @block.scalar
        def _(scalar):
            # bp chunks on the ACT HWDGE ring, in parallel with x on SP's
            for k, n in enumerate(chunks):
                o = offs[k]
                scalar.dma_start(
                    out=bp_all[:, o * W:(o + n) * W],
                    in_=bpk[:, o * W:(o + n) * W],
                ).then_inc(s_bp[k], 16)

        @block.sync
        def _(sync):
            for k, n in enumerate(chunks):
                o = offs[k]
                sync.dma_start(
                    out=x_all[:, o * T * S:(o + n) * T * S].rearrange(
                        "p (i w) -> p i w", i=n),
                    in_=x_r[:, o:o + n, :],
                ).then_inc(s_x[k], 16)
            sync.wait_ge(s_oh, NT + 1)
            sync.dma_start(out=out[:], in_=out_sb[:]).then_inc(s_bp[0], 16)

        @block.vector
        def _(vector):
            for i in range(NT):
                if i == offs[chunk_of[i]]:
                    vector.wait_ge(s_bp[chunk_of[i]], 16)
                if i >= 2:
                    # oh half (i % 2) is reused: wait for tile i-2's matmuls
                    vector.wait_ge(s_pe, i - 1)
                bp_ap = bp_all[:, i * W:i * W + T]
                bp_bcast = bass.AP(
                    tensor=bp_ap.tensor,
                    offset=bp_ap.offset,
                    ap=[bp_ap.ap[0], [bp_ap.ap[1][0], T], [0, C]],
                )
                io_ap = bp_all[:, i * W + T:i * W + T + C]
                io_bcast = bass.AP(
                    tensor=io_ap.tensor,
                    offset=io_ap.offset,
                    ap=[io_ap.ap[0], [0, T], io_ap.ap[1]],
                )
                half = (i % 2) * T * C
                oh3 = oh2[:, half:half + T * C].rearrange(
                    "p (t c) -> p t c", t=T, c=C)
                vector.tensor_tensor(
                    out=oh3, in0=bp_bcast, in1=io_bcast,
                    op=mybir.AluOpType.is_equal,
                ).then_inc(s_oh, 1)
            vector.wait_ge(s_pe, NT)
            vector.tensor_copy(out=out_sb[:], in_=psum_t[:]).then_inc(s_oh, 1)

        @block.tensor
        def _(tensor):
            for i in range(NT):
                if i == offs[chunk_of[i]]:
                    tensor.wait_ge(s_x[chunk_of[i]], 16)
                tensor.wait_ge(s_oh, i + 1)
                xcol = i * T * S
                half = (i % 2) * T * C
                for j in range(J):
                    first = i == 0 and j == 0
                    last = i == NT - 1 and j == J - 1
                    mm = tensor.matmul(
                        out=psum_t[:],
                        lhsT=x_all[:, xcol + j * (G * S):
                                   xcol + (j + 1) * (G * S)],
                        rhs=oh2[:, half + j * (G * C):
                                half + (j + 1) * (G * C)],
                        start=first, stop=last,
                    )
                    if j == J - 1:
                        mm.then_inc(s_pe, 1)
    return nc


_CACHE = {}


def _get_nc(R, T, NT, chunks):
    key = (R, T, NT, tuple(chunks))
    if key not in _CACHE:
        _CACHE[key] = build_nc(R, T, NT, chunks)
    return _CACHE[key]


def pack_bp(bpf_shard, T, NT):
    """[R] f32 -> [P, NT*(T+C)] f32 matching the x layout.

    x slot (p, i*T + t) holds row i*(P*T) + p*T + t; bp uses the same
    permutation, with iota(C) appended per compute tile.
    """
    R = bpf_shard.shape[0]
    assert R == NT * P * T
    perm = bpf_shard.reshape(NT, P, T).transpose(1, 0, 2)
    out = np.empty((P, NT, T + C), np.float32)
    out[:, :, :T] = perm
    out[:, :, T:] = np.arange(C, dtype=np.float32)
    return np.ascontiguousarray(out.reshape(P, NT * (T + C)))


def finish_host(partials):
    """partials: list of [P, G*C] f32 per-core PSUM drains -> scalar loss."""
    acc = np.zeros((P, G * C), np.float64)
    for r in partials:
        acc += r.astype(np.float64)
    cs_T = np.zeros((S, C), np.float64)
    for j in range(G):
        cs_T += acc[j * S:(j + 1) * S, j * C:(j + 1) * C]
    class_sums = cs_T.T                      # [C, S]
    colsum = class_sums.sum(axis=0)          # == output.sum(0)
    demP = class_sums / colsum
    i0, i1 = np.triu_indices(S, k=1)
    dpgs = (demP[:, i0] - demP[:, i1]) ** 2
    loss = dpgs.sum() / (C * i0.shape[0])
    return np.asarray(-loss, dtype=np.float32)


def run_device(x, bpf, trace=False, **trace_kwargs):
    """x: [N, 8] f32, bpf: [N] f32 (integer-valued). Returns BassKernelResults."""
    from concourse.bass_utils import run_bass_kernel_spmd

    N = x.shape[0]
    assert N % (NCORES * P * T) == 0, N
    R = N // NCORES
    NT_ = R // (P * T)
    in_maps = [
        {"x": x[c * R:(c + 1) * R],
         "bp": pack_bp(bpf[c * R:(c + 1) * R], T, NT_)}
        for c in range(NCORES)
    ]
    nc = _get_nc(R, T, NT_, CHUNKS)
    return run_bass_kernel_spmd(
        nc, in_maps, core_ids=list(range(NCORES)), trace=trace, **trace_kwargs
    )


def kernel(output, biased_predictions, labels=None, num_classes=10,
           num_subgroups=8, **_ignored):
    assert int(num_classes) == C and int(num_subgroups) == S
    x = np.ascontiguousarray(np.asarray(output), dtype=np.float32)
    bp = np.asarray(biased_predictions)
    bpf = np.ascontiguousarray(bp.astype(np.float32))
    res = run_device(x, bpf)
    return finish_host([r["out"] for r in res.results])


# revision 30
# speedup vs baseline: 1.3390x; 1.2762x over previous
"""Trainium2 Bass kernel for nn_DemographicParityGap.

reference:
    class_sums[c, s] = sum_{n: bp[n]==c} output[n, s]        # segment sum, [C, S]
    demP = class_sums / output.sum(0)                        # [C, S]
    loss = mean over (c, pairs) of (demP[:, i0] - demP[:, i1])**2
    return -loss

Strategy (data-parallel over the 8 NeuronCores, hint-compliant):
  - Shard N rows across 8 cores.  Each core computes a partial per-(class,
    subgroup) sum; column sums are recovered as class_sums.sum(0) (every row
    belongs to exactly one class), so only one tiny [128, 160] partial per
    core leaves the device.  The host sums the 8 partials (the "all-reduce"
    of the tiny tensor) and finishes the pairwise-gap math.

  Device-side segment sum via one-hot matmuls, batched 16 row-groups per
  matmul so the PE stays off the instruction-issue floor:
    - x layout [128, T*8]: partition p holds rows (p*T + t), t<T, each
      row's 8 subgroup values contiguous.
    - one-hot tile [128, T*10] built by a single DVE is_equal against an
      iota constant packed next to bp in the same preloaded tensor.
    - per 16-group supergroup j: matmul(lhsT = x[:, 128j:128(j+1)] (16
      groups x 8 subgroups), rhs = onehot[:, 160j:160(j+1)] (16 groups x
      10 classes)) -> PSUM [128, 160].  Diagonal 8x10 blocks (t==g) are the
      per-class partial sums; off-diagonal blocks are ignored.  All
      supergroups accumulate into one PSUM tile (start on first, stop on
      last), drained once per core.

  This toolchain's walrus codegen allows exactly ONE sync-wait command per
  instruction (TT/LW/DMA structs alike), which dictates the sync shape:
    - bp+iota preloaded in one DMA; all is_equal ops share that single
      observed dependency.
    - a tiny DVE "observer" copy re-reads the newest one-hot tile before
      each is_equal so the is_equal carries only the PE buffer-release wait.
    - a 1x1 dummy matmul reading only x absorbs the x-DMA wait, so the
      first real matmul of a tile waits only on the DVE one-hot.
    - at most 8 DMAs total (1 bp + NX x-chunks + 1 out), one per DMAHW sem
      lane, so no DMA carries a lane-reuse wait on top of a data wait.
"""

import numpy as np

P = 128          # partitions
C = 10           # num classes
S = 8            # num subgroups
G = 16           # row-groups (of 128 rows each) per matmul; G*S == 128
NCORES = 8

N_FULL = 4_194_304
T = 256          # row-groups per partition per compute tile
NT = 16          # compute tiles per core; R = NT*P*T rows per core
CHUNKS = (1,) * 16         # DMA chunk sizes in tiles (sum == NT).  Per-tile
                           # chunks keep PE gating fine-grained; the ~1.2us
                           # HWDGE issue cost per DMA overlaps the data
                           # stream (2.9us/MiB), and bp DMAs ride the
                           # scalar-engine ring in parallel.


def build_nc(R, T, NT, chunks):
    """Raw-Bass (no TileContext) pipeline.

    This walrus build allows exactly ONE sync-wait command per instruction;
    Tile's auto-sems routinely embed several (and its tail drain aggregates
    all procs), which fails codegen.  Raw Bass emits every wait as its own
    standalone instruction, which is always legal.

    Engine programs:
      SP (sync):  bp DMA, NX x-chunk DMAs (each -> own sem), final out DMA.
      DVE:        per tile: is_equal one-hot into half of a double buffer,
                  gated on bp DMA and (for reuse) PE tile completions;
                  final PSUM->SBUF drain copy.
      PE:         per tile: J matmuls accumulating into one PSUM tile,
                  gated on the x chunk's DMA sem and the DVE one-hot sem.
    """
    from contextlib import ExitStack

    import concourse.bass as bass
    from concourse import mybir

    assert R == NT * P * T
    assert T % G == 0 and sum(chunks) == NT
    J = T // G
    W = T + C        # packed bp tile width: [bp(T), iota(C)]
    NX = len(chunks)
    offs = [sum(chunks[:k]) for k in range(NX)]        # first tile per chunk
    chunk_of = [k for k in range(NX) for _ in range(chunks[k])]
    f32 = mybir.dt.float32

    nc = bass.Bass()
    f32r = mybir.dt.float32r
    x = nc.dram_tensor("x", [R, S], f32r, kind="ExternalInput")
    bpk = nc.dram_tensor("bp", [P, NT * W], f32, kind="ExternalInput")
    out = nc.dram_tensor("out", [P, G * C], f32, kind="ExternalOutput")

    # tile i, partition p: rows i*(P*T) + p*T + t  ->  [P, NT, T*S]
    x_r = x[:].rearrange("(i p t) s -> p i (t s)", i=NT, p=P)

    with ExitStack() as ctx:
        x_all = ctx.enter_context(nc.sbuf_tensor([P, NT * T * S], f32r))
        bp_all = ctx.enter_context(nc.sbuf_tensor([P, NT * W], f32))
        oh2 = ctx.enter_context(nc.sbuf_tensor([P, 2 * T * C], f32r))
        out_sb = ctx.enter_context(nc.sbuf_tensor([P, G * C], f32))
        psum_t = ctx.enter_context(nc.psum_tensor([P, G * C], f32))
        s_bp = [ctx.enter_context(nc.semaphore(f"s_bp{k}")) for k in range(NX)]
        s_x = [ctx.enter_context(nc.semaphore(f"s_x{k}")) for k in range(NX)]
        s_oh = ctx.enter_context(nc.semaphore("s_oh"))
        s_pe = ctx.enter_context(nc.semaphore("s_pe"))
        block = ctx.enter_context(nc.Block(no_gpsimd_drain=True))

        @block.sync
        def _(sync):
            # Interleaved per-chunk bp/x DMAs on one FIFO ring: tiny bp_k
            # first so the DVE one-hot can start while x_k streams.
            for k, n in enumerate(chunks):
                o = offs[k]
                sync.dma_start(
                    out=bp_all[:, o * W:(o + n) * W],
                    in_=bpk[:, o * W:(o + n) * W],
                ).then_inc(s_bp[k], 16)
                sync.dma_start(
                    out=x_all[:, o * T * S:(o + n) * T * S].rearrange(
                        "p (i w) -> p i w", i=n),
                    in_=x_r[:, o:o + n, :],
                ).then_inc(s_x[k], 16)
            sync.wait_ge(s_oh, NT + 1)
            sync.dma_start(out=out[:], in_=out_sb[:]).then_inc(s_bp[0], 16)

        @block.vector
        def _(vector):
            for i in range(NT):
                if i == offs[chunk_of[i]]:
                    vector.wait_ge(s_bp[chunk_of[i]], 16)
                if i >= 2:
                    # oh half (i % 2) is reused: wait for tile i-2's matmuls
                    vector.wait_ge(s_pe, i - 1)
                bp_ap = bp_all[:, i * W:i * W + T]
                bp_bcast = bass.AP(
                    tensor=bp_ap.tensor,
                    offset=bp_ap.offset,
                    ap=[bp_ap.ap[0], [bp_ap.ap[1][0], T], [0, C]],
                )
                io_ap = bp_all[:, i * W + T:i * W + T + C]
                io_bcast = bass.AP(
                    tensor=io_ap.tensor,
                    offset=io_ap.offset,
                    ap=[io_ap.ap[0], [0, T], io_ap.ap[1]],
                )
                half = (i % 2) * T * C
                oh3 = oh2[:, half:half + T * C].rearrange(
                    "p (t c) -> p t c", t=T, c=C)
                vector.tensor_tensor(
                    out=oh3, in0=bp_bcast, in1=io_bcast,
                    op=mybir.AluOpType.is_equal,
                ).then_inc(s_oh, 1)
            vector.wait_ge(s_pe, NT)
            vector.tensor_copy(out=out_sb[:], in_=psum_t[:]).then_inc(s_oh, 1)

        @block.tensor
        def _(tensor):
            for i in range(NT):
                if i == offs[chunk_of[i]]:
                    tensor.wait_ge(s_x[chunk_of[i]], 16)
                tensor.wait_ge(s_oh, i + 1)
                xcol = i * T * S
                half = (i % 2) * T * C
                for j in range(J):
                    first = i == 0 and j == 0
                    last = i == NT - 1 and j == J - 1
                    # float32r: single-pass PE fp32 (plain fp32 runs as two
                    # half-speed passes).  Exact here: every product is
                    # x*1 or x*0, and any uniform input-rounding bias
                    # cancels in the demP ratio.
                    mm = tensor.matmul(
                        out=psum_t[:],
                        lhsT=x_all[:, xcol + j * (G * S):
                                   xcol + (j + 1) * (G * S)],
                        rhs=oh2[:, half + j * (G * C):
                                half + (j + 1) * (G * C)],
                        start=first, stop=last,
                    )
                    if j == J - 1:
                        mm.then_inc(s_pe, 1)
    return nc


_CACHE = {}


def _get_nc(R, T, NT, chunks):
    key = (R, T, NT, tuple(chunks))
    if key not in _CACHE:
        _CACHE[key] = build_nc(R, T, NT, chunks)
    return _CACHE[key]


def pack_bp(bpf_shard, T, NT):
    """[R] f32 -> [P, NT*(T+C)] f32 matching the x layout.

    x slot (p, i*T + t) holds row i*(P*T) + p*T + t; bp uses the same
    permutation, with iota(C) appended per compute tile.
    """
    R = bpf_shard.shape[0]
    assert R == NT * P * T
    perm = bpf_shard.reshape(NT, P, T).transpose(1, 0, 2)
    out = np.empty((P, NT, T + C), np.float32)
    out[:, :, :T] = perm
    out[:, :, T:] = np.arange(C, dtype=np.float32)
    return np.ascontiguousarray(out.reshape(P, NT * (T + C)))


def finish_host(partials):
    """partials: list of [P, G*C] f32 per-core PSUM drains -> scalar loss."""
    acc = np.zeros((P, G * C), np.float64)
    for r in partials:
        acc += r.astype(np.float64)
    cs_T = np.zeros((S, C), np.float64)
    for j in range(G):
        cs_T += acc[j * S:(j + 1) * S, j * C:(j + 1) * C]
    class_sums = cs_T.T                      # [C, S]
    colsum = class_sums.sum(axis=0)          # == output.sum(0)
    demP = class_sums / colsum
    i0, i1 = np.triu_indices(S, k=1)
    dpgs = (demP[:, i0] - demP[:, i1]) ** 2
    loss = dpgs.sum() / (C * i0.shape[0])
    return np.asarray(-loss, dtype=np.float32)


def run_device(x, bpf, trace=False, **trace_kwargs):
    """x: [N, 8] f32, bpf: [N] f32 (integer-valued). Returns BassKernelResults."""
    from concourse.bass_utils import run_bass_kernel_spmd

    N = x.shape[0]
    assert N % (NCORES * P * T) == 0, N
    R = N // NCORES
    NT_ = R // (P * T)
    in_maps = [
        {"x": x[c * R:(c + 1) * R],
         "bp": pack_bp(bpf[c * R:(c + 1) * R], T, NT_)}
        for c in range(NCORES)
    ]
    nc = _get_nc(R, T, NT_, CHUNKS)
    return run_bass_kernel_spmd(
        nc, in_maps, core_ids=list(range(NCORES)), trace=trace, **trace_kwargs
    )


def kernel(output, biased_predictions, labels=None, num_classes=10,
           num_subgroups=8, **_ignored):
    assert int(num_classes) == C and int(num_subgroups) == S
    x = np.ascontiguousarray(np.asarray(output), dtype=np.float32)
    bp = np.asarray(biased_predictions)
    bpf = np.ascontiguousarray(bp.astype(np.float32))
    res = run_device(x, bpf)
    return finish_host([r["out"] for r in res.results])
